# revision 22
# baseline (speedup 1.0000x reference)
"""Local (sliding-window) attention kernel for TRN2, 8 NeuronCores.

Sharding: core c -> batch b=c//4, head-group hg=c%4 (4 heads of 16).
Each core computes qkv projection for its heads, banded attention, and a
partial out-projection (its heads' columns of Wo). Host sums the 4
partials per batch and adds bo.

v2: all matmul operands bf16 (2x moving-operand stream rate vs fp32r,
keeps the PE HAM clock-gate warm), softmax normalization via
reciprocal_approx_fast + rank-1 ones-matmul partition broadcast
(replaces the 4us vector.reciprocal + 1.2us gpsimd broadcast serial
chain), bf16 output DMA (host upcasts + sums partials).

Device algorithm (per core):
  qkT[512,2048]  = wqk.T @ xT          (Q rows pre-scaled by 1/sqrt(hd))
  V  [2048,260]  = xT.T @ wv           (token-major; +bias, with a ones
                                        column per head -> vaug)
  per head h, key-block j (128 keys):
    S^T[k,q]     = kT_hj.T @ qT (q-window = 384 cols: blocks j..j+2)
    P^T          = exp(S^T) (bf16), zero band-complement triangles
    yT_psum[65,512] += vaug_hj.T @ P^T   (row 64 = softmax denominator)
  per (h, q-range g of 512):
    rec[1,512] = approx 1/denom (DVE custom op, from PSUM)
    bc[64,512] = ones[1,64].T @ rec      (PE rank-1 broadcast)
    yT (bf16)  = yT_psum * bc
  out[2048,1024] = yT.T @ wo  (partial; host adds across head-groups + bo)
"""

import os
import sys

import numpy as np

if "/opt/trn_rl_repo" not in sys.path:
    sys.path.insert(0, "/opt/trn_rl_repo")

B, T, D = 2, 2048, 1024
H, W = 16, 256
HD = D // H          # 64
NCORES = 8
HPC = 4              # heads per core
FB = HPC * HD        # 256 f-columns per core

_STATE: dict = {}


def _build_module():
    import concourse.bacc as bacc
    import concourse.tile as tile
    from concourse import mybir

    dt = mybir.dt
    AF = mybir.ActivationFunctionType
    OP = mybir.AluOpType

    nc = bacc.Bacc(
        "TRN2",
        target_bir_lowering=False,
        debug=False,
        enable_asserts=False,
        num_devices=NCORES,
    )

    f32 = dt.float32
    f32r = dt.float32r
    bf16 = dt.bfloat16
    xT_d = nc.dram_tensor("xT", [D, T], bf16, kind="ExternalInput").ap()
    wqk_d = nc.dram_tensor("wqk", [D, 2 * FB], bf16, kind="ExternalInput").ap()
    bqk_d = nc.dram_tensor("bqk", [128, 4], f32, kind="ExternalInput").ap()
    wv_d = nc.dram_tensor("wv", [D, FB], bf16, kind="ExternalInput").ap()
    bvb_d = nc.dram_tensor("bvb", [128, HPC, HD], f32, kind="ExternalInput").ap()
    wo_d = nc.dram_tensor("wo", [FB, D], bf16, kind="ExternalInput").ap()
    tris_d = nc.dram_tensor("tris", [128, 256], bf16, kind="ExternalInput").ap()
    out_d = nc.dram_tensor("out_p", [T, D], bf16, kind="ExternalOutput").ap()

    KC = D // 128     # 8 contraction chunks
    NT = T // 128     # 16 token tiles / key blocks
    NQ = T // 512     # 4 q-ranges

    with tile.TileContext(nc) as tc:
        with (
            tc.tile_pool(name="const", bufs=1) as cpool,
            tc.tile_pool(name="work", bufs=3) as wpool,
            tc.tile_pool(name="psA", bufs=2, space="PSUM") as ppA,
            tc.tile_pool(name="ps", bufs=4, space="PSUM") as ppool,
        ):
            # ---- persistent SBUF ----
            # per-contraction-chunk tiles so each matmul depends only on
            # its own chunk's DMA, not the whole operand
            xTa_t = [cpool.tile([128, 1024], bf16, name=f"xTa{a}")
                     for a in range(KC)]
            xTb_t = [cpool.tile([128, 1024], bf16, name=f"xTb{a}")
                     for a in range(KC)]
            wqk_t = [cpool.tile([128, 2 * FB], bf16, name=f"wqk{a}")
                     for a in range(KC)]
            wv_t = [cpool.tile([128, FB], bf16, name=f"wv{a}")
                    for a in range(KC)]
            wo_sb = cpool.tile([128, 2, D], bf16)
            bqk_sb = cpool.tile([128, 4], f32)
            bvb_sb = cpool.tile([128, HPC, HD], f32)
            tris_sb = cpool.tile([128, 256], bf16)
            qkT_sb = cpool.tile([128, 4, T], bf16)
            vaug_sb = cpool.tile([128, NT, HPC, HD + 1], bf16)
            yTn_sb = cpool.tile([128, 2, T], bf16)

            # vaug ones columns via memset (DVE is idle during load)
            for h in range(HPC):
                nc.vector.memset(vaug_sb[:, :, h, HD:HD + 1], 1.0)

            # Two DMA queues (SP + ACT hwdge): group-0 operands (wqk, xTa)
            # lead both queues so the first matmul can start ~1us in;
            # constants follow, then second-half operands. ACT is idle
            # during the load phase.
            for a in range(KC):
                nc.sync.dma_start(wqk_t[a][:], wqk_d[a * 128:(a + 1) * 128, :])
                nc.scalar.dma_start(
                    xTa_t[a][:], xT_d[a * 128:(a + 1) * 128, 0:1024]
                )
            nc.sync.dma_start(bqk_sb[:], bqk_d[:])
            nc.sync.dma_start(bvb_sb[:], bvb_d[:])
            nc.sync.dma_start(tris_sb[:], tris_d[:])
            for a in range(KC):
                nc.sync.dma_start(wv_t[a][:], wv_d[a * 128:(a + 1) * 128, :])
            for a in range(KC):
                nc.scalar.dma_start(
                    xTb_t[a][:], xT_d[a * 128:(a + 1) * 128, 1024:2048]
                )
            for f in range(2):
                nc.sync.dma_start(wo_sb[:, f, :], wo_d[f * 128:(f + 1) * 128, :])

            # ---- qkT projection: [512, 2048] ----
            # a-outer in groups of 8 PSUM tiles so the first pass streams
            # with the xT/wqk DMA arrivals instead of serializing one
            # accumulation chain against the whole load.
            def qkT_group(grp, xh_t):
                # m-outer, 2-tile passes: only 2 PSUM banks at a time so
                # the group-1 pass can coexist with the EARLY attention
                # batch's PSUM tiles
                for m in range(4):
                    ps_p = [
                        ppool.tile([128, 512], f32, tag="ps",
                                   name=f"ps_qk{grp}_{m}_{n}")
                        for n in range(2)
                    ]
                    for a in range(KC):
                        for n in range(2):
                            nc.tensor.matmul(
                                ps_p[n][:],
                                lhsT=wqk_t[a][:, m * 128:(m + 1) * 128],
                                rhs=xh_t[a][:, n * 512:(n + 1) * 512],
                                start=(a == 0),
                                stop=(a == KC - 1),
                            )
                    for n in range(2):
                        nc.scalar.activation(
                            qkT_sb[:, m,
                                   (2 * grp + n) * 512:(2 * grp + n + 1) * 512],
                            ps_p[n][:],
                            AF.Identity,
                            bias=bqk_sb[:, m:m + 1],
                        )

            def v_proj(t, xh_t):
                tl = t % 8
                ps_v = ppool.tile([128, HPC, HD], f32, tag="ps", name=f"ps_v_{t}")
                for a in range(KC):
                    nc.tensor.matmul(
                        ps_v[:],
                        lhsT=xh_t[a][:, tl * 128:(tl + 1) * 128],
                        rhs=wv_t[a][:],
                        start=(a == 0),
                        stop=(a == KC - 1),
                    )
                for h in range(HPC):
                    nc.vector.tensor_tensor(
                        out=vaug_sb[:, t, h, 0:HD],
                        in0=ps_v[:, h, :],
                        in1=bvb_sb[:, h, :],
                        op=OP.add,
                    )

            qkT_group(0, xTa_t)
            for t in range(8):
                v_proj(t, xTa_t)

            # ---- attention: software-pipelined over flattened (h, j) ----
            # stage A:  paired S^T matmuls (j even + odd) into a 2-bank
            #           PSUM tile -> ONE exp over both halves (strided AP)
            #           -> band-complement triangle masks per half (DVE)
            # stage B:  pV matmuls
            # Step order interleaves an EARLY batch (all h, j<=3, pV
            # clipped to q-range g=0) before group-1 projection, so ACT's
            # exp work overlaps the PE-dense projection of tokens
            # 1024-2047. The clipped g=1 contributions of j=2,3 replay
            # from retained pT tiles at each head's first REST step.
            # The (h,g)-tail normalize chain (denom copy -> approx recip ->
            # partition broadcast -> multiply) is staggered across later
            # ticks via `post`, so each op's inputs are already complete
            # when it reaches its engine's strict-FIFO head — otherwise
            # the chain head-blocks the DVE/GpSimd queues that the
            # per-step mask ops need, stalling the PE ~13us per group.
            DELAY = 4
            EARLY_N = 4 * HPC
            steps = [(h, j) for h in range(HPC) for j in range(4)]
            steps += [(h, j) for h in range(HPC) for j in range(4, NT)]
            pT_t = {}
            ps_y = {}
            post = {}

            def at_step(s, fn):
                post.setdefault(s, []).append(fn)

            def out_proj(g):
                for mt in range(4 * g, 4 * g + 4):
                    o_sb = wpool.tile(
                        [128, 2, 512], bf16, bufs=4,
                        name=f"o_{mt}", tag="o_sb",
                    )
                    for nn in range(2):
                        ps_o = ppool.tile(
                            [128, 512], f32, tag="ps",
                            name=f"ps_o_{mt}_{nn}",
                        )
                        for fc in range(2):
                            nc.tensor.matmul(
                                ps_o[:],
                                lhsT=yTn_sb[:, fc, mt * 128:(mt + 1) * 128],
                                rhs=wo_sb[:, fc, nn * 512:(nn + 1) * 512],
                                start=(fc == 0),
                                stop=(fc == 1),
                            )
                        if (mt + nn) % 2 == 0:
                            nc.vector.tensor_copy(
                                out=o_sb[:, nn, :], in_=ps_o[:]
                            )
                        else:
                            nc.scalar.copy(o_sb[:, nn, :], ps_o[:])
                    eng = nc.sync if mt % 2 == 0 else nc.gpsimd
                    eng.dma_start(
                        out_d[mt * 128:(mt + 1) * 128, :], o_sb[:]
                    )

            def stage_a(k):
                h, j0 = steps[k]
                if j0 % 2 == 1:
                    return
                po = 64 * (h % 2)
                ps_s2 = ppA.tile([128, 2, 512], f32, tag="ps_s",
                                 name=f"ps_s_{k}")
                pT2 = wpool.tile([128, 2, 384], bf16, bufs=10,
                                 name=f"pT_{k}", tag="pT")
                for u in range(2):
                    j = j0 + u
                    qw = min(384, T - 128 * j)
                    nc.tensor.matmul(
                        ps_s2[:, u, 0:qw],
                        lhsT=qkT_sb[po:po + 64, 2 + h // 2,
                                    j * 128:(j + 1) * 128],
                        rhs=qkT_sb[po:po + 64, h // 2,
                                   j * 128:j * 128 + qw],
                        start=True,
                        stop=True,
                        skip_group_check=True,
                    )
                nc.scalar.activation(pT2[:, :, :], ps_s2[:, :, 0:384],
                                     AF.Exp)
                for u in range(2):
                    j = j0 + u
                    view = pT2[:, u, :]
                    if min(384, T - 128 * j) == 384:
                        pv = view.rearrange(
                            "p (a b) -> p a b", a=3)[:, 0:3:2, :]
                        tv = tris_sb[:].rearrange("p (a b) -> p a b", a=2)
                        nc.vector.tensor_tensor(
                            out=pv, in0=pv, in1=tv, op=OP.mult,
                        )
                    else:
                        nc.vector.tensor_tensor(
                            out=view[:, 0:128], in0=view[:, 0:128],
                            in1=tris_sb[:, 0:128], op=OP.mult,
                        )
                    pT_t[k + u] = view

            def stage_b(idx, pidx):
                h, j = steps[idx]
                po = 64 * (h % 2)
                early = idx < EARLY_N
                qwin = min(384, T - 128 * j)
                if j == 4:
                    # replay the g=1 parts of j=2,3 clipped during EARLY
                    for jr in (2, 3):
                        pTr = pT_t.pop(h * 4 + jr)
                        c0 = 512 - 128 * jr
                        if (h, 1) not in ps_y:
                            ps_y[(h, 1)] = ppool.tile(
                                [65, 512], f32, tag="ps",
                                name=f"ps_y_{h}_1",
                            )
                        nc.tensor.matmul(
                            ps_y[(h, 1)][:, 0:384 - c0],
                            lhsT=vaug_sb[:, jr, h, :],
                            rhs=pTr[:, c0:384],
                            start=(jr == 2),
                            stop=False,
                            skip_group_check=True,
                        )
                pT = pT_t[idx]
                if not (early and j >= 2):
                    del pT_t[idx]
                gs = [0] if early else range(
                    (128 * j) // 512, (128 * j + qwin - 1) // 512 + 1)
                for g in gs:
                    c0 = max(0, 512 * g - 128 * j)
                    c1 = min(qwin, 512 * (g + 1) - 128 * j)
                    if (h, g) not in ps_y:
                        ps_y[(h, g)] = ppool.tile(
                            [65, 512], f32, tag="ps", name=f"ps_y_{h}_{g}"
                        )
                    first = (j == max(0, 4 * g - 2))
                    last = (j == min(NT - 1, 4 * g + 3))
                    d0 = 128 * j + c0 - 512 * g
                    nc.tensor.matmul(
                        ps_y[(h, g)][:, d0:d0 + (c1 - c0)],
                        lhsT=vaug_sb[:, j, h, :],
                        rhs=pT[:, c0:c1],
                        start=first,
                        stop=last,
                        skip_group_check=True,
                    )
                    if not last:
                        continue
                    yps = ps_y.pop((h, g))
                    # reciprocal_approx_fast and partition_broadcast both
                    # require partition base 0: stage the denominator row
                    # at partition 0 first.
                    dn = wpool.tile([1, 512], f32, bufs=4,
                                    name=f"dn_{h}_{g}", tag="dn")
                    rec = wpool.tile([1, 512], f32, bufs=4,
                                     name=f"rec_{h}_{g}", tag="rec")
                    bc_sb = wpool.tile([64, 512], f32, bufs=3,
                                       name=f"bc_{h}_{g}", tag="bc")

                    def dn_copy(dn=dn, yps=yps):
                        nc.scalar.copy(dn[:], yps[64:65, :])

                    def do_recip(rec=rec, dn=dn):
                        nc.vector.reciprocal_approx_fast(rec[:], dn[:])

                    def do_bcast(bc_sb=bc_sb, rec=rec):
                        nc.gpsimd.partition_broadcast(bc_sb[:], rec[0:1, :])

                    def do_mult(yps=yps, bc_sb=bc_sb, po=po, h=h, g=g):
                        nc.vector.tensor_tensor(
                            out=yTn_sb[po:po + 64, h // 2,
                                       g * 512:(g + 1) * 512],
                            in0=yps[0:64, :],
                            in1=bc_sb[:],
                            op=OP.mult,
                        )

                    at_step(pidx + 1, dn_copy)
                    at_step(pidx + 2, do_recip)
                    at_step(pidx + 3, do_bcast)
                    at_step(pidx + 4, do_mult)
                    if h == HPC - 1:
                        at_step(pidx + 4, lambda g=g: out_proj(g))

            emits = [lambda: qkT_group(1, xTb_t)]
            emits += [(lambda t=t: v_proj(t, xTb_t)) for t in range(8, NT)]
            n_emit = len(emits)
            total_ticks = EARLY_N + n_emit + (len(steps) - EARLY_N) \
                + DELAY + 10
            a_i = 0
            b_i = 0
            for tick in range(total_ticks):
                emit_phase = EARLY_N <= tick < EARLY_N + n_emit
                if tick < EARLY_N:
                    stage_a(a_i)
                    a_i += 1
                elif emit_phase:
                    emits[tick - EARLY_N]()
                elif a_i < len(steps):
                    stage_a(a_i)
                    a_i += 1
                if b_i < a_i and (a_i - b_i > DELAY or emit_phase
                                  or a_i == len(steps)):
                    stage_b(b_i, tick)
                    b_i += 1
                for fn in post.pop(tick, []):
                    fn()

    nc.compile()
    from concourse.bass_interp import get_hw_module

    nc.m = get_hw_module(nc.m)
    return nc


def _shard_inputs(x, Wqkv, bqkv, Wo, bo):
    import ml_dtypes

    bfdt = ml_dtypes.bfloat16

    x = np.asarray(x, np.float32)
    Wqkv = np.asarray(Wqkv, np.float32)
    bqkv = np.asarray(bqkv, np.float32)
    Wo = np.asarray(Wo, np.float32)

    scale = 1.0 / np.sqrt(np.float32(HD))
    c_idx = np.arange(128)[:, None]
    u_idx = np.arange(128)[None, :]
    tri0 = (u_idx >= c_idx).astype(np.float32)   # keys block j vs q block j
    tri1 = (u_idx < c_idx).astype(np.float32)    # keys block j vs q block j+2
    tris = np.concatenate([tri0, tri1], axis=1)

    in_maps = []
    for c in range(NCORES):
        b, hg = divmod(c, HPC)
        r0 = hg * FB
        Wq = Wqkv[r0:r0 + FB] * scale
        Wk = Wqkv[D + r0:D + r0 + FB]
        Wv = Wqkv[2 * D + r0:2 * D + r0 + FB]
        bq = bqkv[r0:r0 + FB] * scale
        bk = bqkv[D + r0:D + r0 + FB]
        bv = bqkv[2 * D + r0:2 * D + r0 + FB]
        in_maps.append({
            "xT": np.ascontiguousarray(x[b].T).astype(bfdt),
            "wqk": np.ascontiguousarray(
                np.concatenate([Wq, Wk], 0).T).astype(bfdt),
            "bqk": np.ascontiguousarray(
                np.concatenate([bq, bk]).reshape(4, 128).T),
            "wv": np.ascontiguousarray(Wv.T).astype(bfdt),
            "bvb": np.ascontiguousarray(
                np.broadcast_to(bv[None, :], (128, FB))
            ).reshape(128, HPC, HD),
            "wo": np.ascontiguousarray(Wo[:, r0:r0 + FB].T).astype(bfdt),
            "tris": tris.astype(bfdt),
            "vone": np.ones((128, 64), bfdt),
        })
    return in_maps


def kernel(x, Wqkv, bqkv, Wo, bo):
    from concourse import bass_utils

    if "nc" not in _STATE:
        _STATE["nc"] = _build_module()
    nc = _STATE["nc"]

    in_maps = _shard_inputs(x, Wqkv, bqkv, Wo, bo)
    trace = bool(os.environ.get("TRNKERN_TRACE"))
    res = bass_utils.run_bass_kernel_spmd(
        nc,
        in_maps,
        core_ids=list(range(NCORES)),
        trace=trace,
    )
    _STATE["last"] = res

    bo = np.asarray(bo, np.float32)
    out = np.empty((B, T, D), np.float32)
    for b in range(B):
        acc = res.results[b * HPC]["out_p"].astype(np.float32)
        for hg in range(1, HPC):
            acc = acc + res.results[b * HPC + hg]["out_p"].astype(np.float32)
        out[b] = acc + bo[None, :]
    return out


# revision 26
# speedup vs baseline: 1.1850x; 1.1850x over previous
"""Local (sliding-window) attention kernel for TRN2, 8 NeuronCores.

Sharding: core c -> batch b=c//4, head-group hg=c%4 (4 heads of 16).
Each core computes qkv projection for its heads, banded attention, and a
partial out-projection (its heads' columns of Wo). Host sums the 4
partials per batch and adds bo.

v2: all matmul operands bf16 (2x moving-operand stream rate vs fp32r,
keeps the PE HAM clock-gate warm), softmax normalization via
reciprocal_approx_fast + rank-1 ones-matmul partition broadcast
(replaces the 4us vector.reciprocal + 1.2us gpsimd broadcast serial
chain), bf16 output DMA (host upcasts + sums partials).

Device algorithm (per core):
  qkT[512,2048]  = wqk.T @ xT          (Q rows pre-scaled by 1/sqrt(hd))
  V  [2048,260]  = xT.T @ wv           (token-major; +bias, with a ones
                                        column per head -> vaug)
  per head h, key-block j (128 keys):
    S^T[k,q]     = kT_hj.T @ qT (q-window = 384 cols: blocks j..j+2)
    P^T          = exp(S^T) (bf16), zero band-complement triangles
    yT_psum[65,512] += vaug_hj.T @ P^T   (row 64 = softmax denominator)
  per (h, q-range g of 512):
    rec[1,512] = approx 1/denom (DVE custom op, from PSUM)
    bc[64,512] = ones[1,64].T @ rec      (PE rank-1 broadcast)
    yT (bf16)  = yT_psum * bc
  out[2048,1024] = yT.T @ wo  (partial; host adds across head-groups + bo)
"""

import os
import sys

import numpy as np

if "/opt/trn_rl_repo" not in sys.path:
    sys.path.insert(0, "/opt/trn_rl_repo")

B, T, D = 2, 2048, 1024
H, W = 16, 256
HD = D // H          # 64
NCORES = 8
HPC = 4              # heads per core
FB = HPC * HD        # 256 f-columns per core

_STATE: dict = {}


def _build_module():
    import concourse.bacc as bacc
    import concourse.tile as tile
    from concourse import mybir

    dt = mybir.dt
    AF = mybir.ActivationFunctionType
    OP = mybir.AluOpType

    nc = bacc.Bacc(
        "TRN2",
        target_bir_lowering=False,
        debug=False,
        enable_asserts=False,
        num_devices=NCORES,
    )

    f32 = dt.float32
    f32r = dt.float32r
    bf16 = dt.bfloat16
    xT_d = nc.dram_tensor("xT", [D, T], bf16, kind="ExternalInput").ap()
    wqk_d = nc.dram_tensor("wqk", [D, 2 * FB], bf16, kind="ExternalInput").ap()
    bqk_d = nc.dram_tensor("bqk", [128, 4], f32, kind="ExternalInput").ap()
    wv_d = nc.dram_tensor("wv", [D, FB], bf16, kind="ExternalInput").ap()
    bvb_d = nc.dram_tensor("bvb", [128, HPC, HD], f32, kind="ExternalInput").ap()
    wo_d = nc.dram_tensor("wo", [FB, D], bf16, kind="ExternalInput").ap()
    tris_d = nc.dram_tensor("tris", [128, 256], bf16, kind="ExternalInput").ap()
    out_d = nc.dram_tensor("out_p", [T, D], bf16, kind="ExternalOutput").ap()

    KC = D // 128     # 8 contraction chunks
    NT = T // 128     # 16 token tiles / key blocks
    NQ = T // 512     # 4 q-ranges

    with tile.TileContext(nc) as tc:
        with (
            tc.tile_pool(name="const", bufs=1) as cpool,
            tc.tile_pool(name="work", bufs=3) as wpool,
            tc.tile_pool(name="ps", bufs=8, space="PSUM") as ppool,
        ):
            # ---- persistent SBUF ----
            # per-contraction-chunk tiles so each matmul depends only on
            # its own chunk's DMA, not the whole operand
            xTa_t = [cpool.tile([128, 1024], bf16, name=f"xTa{a}")
                     for a in range(KC)]
            xTb_t = [cpool.tile([128, 1024], bf16, name=f"xTb{a}")
                     for a in range(KC)]
            wqk_t = [cpool.tile([128, 2 * FB], bf16, name=f"wqk{a}")
                     for a in range(KC)]
            wv_t = [cpool.tile([128, FB], bf16, name=f"wv{a}")
                    for a in range(KC)]
            wo_sb = cpool.tile([128, 2, D], bf16)
            bqk_sb = cpool.tile([128, 4], f32)
            bvb_sb = cpool.tile([128, HPC, HD], f32)
            tris_sb = cpool.tile([128, 256], bf16)
            qkT_sb = cpool.tile([128, 4, T], bf16)
            vaug_sb = cpool.tile([128, NT, HPC, HD + 1], bf16)
            yTn_sb = cpool.tile([128, 2, T], bf16)

            # vaug ones columns via memset (DVE is idle during load)
            for h in range(HPC):
                nc.vector.memset(vaug_sb[:, :, h, HD:HD + 1], 1.0)

            # Two DMA queues (SP + ACT hwdge): group-0 operands (wqk, xTa)
            # lead both queues so the first matmul can start ~1us in;
            # constants follow, then second-half operands. ACT is idle
            # during the load phase.
            for a in range(KC):
                nc.sync.dma_start(wqk_t[a][:], wqk_d[a * 128:(a + 1) * 128, :])
                nc.scalar.dma_start(
                    xTa_t[a][:], xT_d[a * 128:(a + 1) * 128, 0:1024]
                )
            nc.sync.dma_start(bqk_sb[:], bqk_d[:])
            nc.sync.dma_start(bvb_sb[:], bvb_d[:])
            nc.sync.dma_start(tris_sb[:], tris_d[:])
            for a in range(KC):
                nc.sync.dma_start(wv_t[a][:], wv_d[a * 128:(a + 1) * 128, :])
            for a in range(KC):
                nc.gpsimd.dma_start(
                    xTb_t[a][:], xT_d[a * 128:(a + 1) * 128, 1024:2048]
                )
            for f in range(2):
                nc.sync.dma_start(wo_sb[:, f, :], wo_d[f * 128:(f + 1) * 128, :])

            # ---- qkT projection: [512, 2048] ----
            # a-outer in groups of 8 PSUM tiles so the first pass streams
            # with the xT/wqk DMA arrivals instead of serializing one
            # accumulation chain against the whole load.
            def qkT_group(grp, xh_t):
                # m-pair passes of 4 PSUM tiles: enough bank distance to
                # avoid same-bank accumulation hazards, while leaving 4
                # ring slots for the EARLY attention batch to coexist
                # with the group-1 pass
                for mm in (0, 2):
                    tiles = [(m, n) for m in (mm, mm + 1) for n in range(2)]
                    ps_g = {
                        mn: ppool.tile([128, 512], f32, tag="ps",
                                       name=f"ps_qk{grp}_{mn[0]}_{mn[1]}")
                        for mn in tiles
                    }
                    for a in range(KC):
                        for (m, n) in tiles:
                            nc.tensor.matmul(
                                ps_g[(m, n)][:],
                                lhsT=wqk_t[a][:, m * 128:(m + 1) * 128],
                                rhs=xh_t[a][:, n * 512:(n + 1) * 512],
                                start=(a == 0),
                                stop=(a == KC - 1),
                            )
                    for (m, n) in tiles:
                        nc.scalar.activation(
                            qkT_sb[:, m,
                                   (2 * grp + n) * 512:(2 * grp + n + 1) * 512],
                            ps_g[(m, n)][:],
                            AF.Identity,
                            bias=bqk_sb[:, m:m + 1],
                        )

            def v_proj(t, xh_t):
                tl = t % 8
                ps_v = ppool.tile([128, HPC, HD], f32, tag="ps", name=f"ps_v_{t}")
                for a in range(KC):
                    nc.tensor.matmul(
                        ps_v[:],
                        lhsT=xh_t[a][:, tl * 128:(tl + 1) * 128],
                        rhs=wv_t[a][:],
                        start=(a == 0),
                        stop=(a == KC - 1),
                    )
                for h in range(HPC):
                    nc.vector.tensor_tensor(
                        out=vaug_sb[:, t, h, 0:HD],
                        in0=ps_v[:, h, :],
                        in1=bvb_sb[:, h, :],
                        op=OP.add,
                    )

            qkT_group(0, xTa_t)
            for t in range(8):
                v_proj(t, xTa_t)

            # ---- attention: software-pipelined over flattened (h, j) ----
            # stage A:  paired S^T matmuls (j even + odd) into a 2-bank
            #           PSUM tile -> ONE exp over both halves (strided AP)
            #           -> band-complement triangle masks per half (DVE)
            # stage B:  pV matmuls
            # Step order interleaves an EARLY batch (all h, j<=3, pV
            # clipped to q-range g=0) before group-1 projection, so ACT's
            # exp work overlaps the PE-dense projection of tokens
            # 1024-2047. The clipped g=1 contributions of j=2,3 replay
            # from retained pT tiles at each head's first REST step.
            # The (h,g)-tail normalize chain (denom copy -> approx recip ->
            # partition broadcast -> multiply) is staggered across later
            # ticks via `post`, so each op's inputs are already complete
            # when it reaches its engine's strict-FIFO head — otherwise
            # the chain head-blocks the DVE/GpSimd queues that the
            # per-step mask ops need, stalling the PE ~13us per group.
            DELAY = 4
            EARLY_N = 4 * HPC
            steps = [(h, j) for h in range(HPC) for j in range(4)]
            steps += [(h, j) for h in range(HPC) for j in range(4, NT)]
            pT_t = {}
            ps_y = {}
            post = {}

            def at_step(s, fn):
                post.setdefault(s, []).append(fn)

            def out_proj(g):
                for mt in range(4 * g, 4 * g + 4):
                    o_sb = wpool.tile(
                        [128, 2, 512], bf16, bufs=4,
                        name=f"o_{mt}", tag="o_sb",
                    )
                    for nn in range(2):
                        ps_o = ppool.tile(
                            [128, 512], f32, tag="ps",
                            name=f"ps_o_{mt}_{nn}",
                        )
                        for fc in range(2):
                            nc.tensor.matmul(
                                ps_o[:],
                                lhsT=yTn_sb[:, fc, mt * 128:(mt + 1) * 128],
                                rhs=wo_sb[:, fc, nn * 512:(nn + 1) * 512],
                                start=(fc == 0),
                                stop=(fc == 1),
                            )
                        if (mt + nn) % 2 == 0:
                            nc.vector.tensor_copy(
                                out=o_sb[:, nn, :], in_=ps_o[:]
                            )
                        else:
                            nc.scalar.copy(o_sb[:, nn, :], ps_o[:])
                    eng = nc.sync if mt % 2 == 0 else nc.gpsimd
                    eng.dma_start(
                        out_d[mt * 128:(mt + 1) * 128, :], o_sb[:]
                    )

            def stage_a(k):
                h, j = steps[k]
                po = 64 * (h % 2)
                qwin = min(384, T - 128 * j)
                ps_s = ppool.tile([128, 384], f32, tag="ps",
                                  name=f"ps_s_{k}")
                nc.tensor.matmul(
                    ps_s[:, :qwin],
                    lhsT=qkT_sb[po:po + 64, 2 + h // 2,
                                j * 128:(j + 1) * 128],
                    rhs=qkT_sb[po:po + 64, h // 2,
                               j * 128:j * 128 + qwin],
                    start=True,
                    stop=True,
                )
                pT = wpool.tile([128, 384], bf16, bufs=14,
                                name=f"pT_{k}", tag="pT")
                nc.scalar.activation(pT[:, :qwin], ps_s[:, :qwin], AF.Exp)
                if qwin == 384:
                    pv = pT[:].rearrange("p (a b) -> p a b", a=3)[:, 0:3:2, :]
                    tv = tris_sb[:].rearrange("p (a b) -> p a b", a=2)
                    nc.vector.tensor_tensor(
                        out=pv, in0=pv, in1=tv, op=OP.mult,
                    )
                else:
                    nc.vector.tensor_tensor(
                        out=pT[:, 0:128], in0=pT[:, 0:128],
                        in1=tris_sb[:, 0:128], op=OP.mult,
                    )
                pT_t[k] = pT

            def stage_b(idx, pidx):
                h, j = steps[idx]
                po = 64 * (h % 2)
                early = idx < EARLY_N
                qwin = min(384, T - 128 * j)
                if j == 4:
                    # replay the g=1 parts of j=2,3 clipped during EARLY
                    for jr in (2, 3):
                        pTr = pT_t.pop(h * 4 + jr)
                        c0 = 512 - 128 * jr
                        if (h, 1) not in ps_y:
                            ps_y[(h, 1)] = ppool.tile(
                                [65, 512], f32, tag="ps",
                                name=f"ps_y_{h}_1",
                            )
                        nc.tensor.matmul(
                            ps_y[(h, 1)][:, 0:384 - c0],
                            lhsT=vaug_sb[:, jr, h, :],
                            rhs=pTr[:, c0:384],
                            start=(jr == 2),
                            stop=False,
                            skip_group_check=True,
                        )
                pT = pT_t[idx]
                if not (early and j >= 2):
                    del pT_t[idx]
                gs = [0] if early else range(
                    (128 * j) // 512, (128 * j + qwin - 1) // 512 + 1)
                for g in gs:
                    c0 = max(0, 512 * g - 128 * j)
                    c1 = min(qwin, 512 * (g + 1) - 128 * j)
                    if (h, g) not in ps_y:
                        ps_y[(h, g)] = ppool.tile(
                            [65, 512], f32, tag="ps", name=f"ps_y_{h}_{g}"
                        )
                    first = (j == max(0, 4 * g - 2))
                    last = (j == min(NT - 1, 4 * g + 3))
                    d0 = 128 * j + c0 - 512 * g
                    nc.tensor.matmul(
                        ps_y[(h, g)][:, d0:d0 + (c1 - c0)],
                        lhsT=vaug_sb[:, j, h, :],
                        rhs=pT[:, c0:c1],
                        start=first,
                        stop=last,
                        skip_group_check=True,
                    )
                    if not last:
                        continue
                    yps = ps_y.pop((h, g))
                    # reciprocal_approx_fast and partition_broadcast both
                    # require partition base 0: stage the denominator row
                    # at partition 0 first.
                    dn = wpool.tile([1, 512], f32, bufs=4,
                                    name=f"dn_{h}_{g}", tag="dn")
                    rec = wpool.tile([1, 512], f32, bufs=4,
                                     name=f"rec_{h}_{g}", tag="rec")
                    bc_sb = wpool.tile([64, 512], f32, bufs=3,
                                       name=f"bc_{h}_{g}", tag="bc")

                    def dn_copy(dn=dn, yps=yps):
                        nc.scalar.copy(dn[:], yps[64:65, :])

                    def do_recip(rec=rec, dn=dn):
                        nc.vector.reciprocal_approx_fast(rec[:], dn[:])

                    def do_bcast(bc_sb=bc_sb, rec=rec):
                        nc.gpsimd.partition_broadcast(bc_sb[:], rec[0:1, :])

                    def do_mult(yps=yps, bc_sb=bc_sb, po=po, h=h, g=g):
                        nc.vector.tensor_tensor(
                            out=yTn_sb[po:po + 64, h // 2,
                                       g * 512:(g + 1) * 512],
                            in0=yps[0:64, :],
                            in1=bc_sb[:],
                            op=OP.mult,
                        )

                    at_step(pidx + 1, dn_copy)
                    at_step(pidx + 2, do_recip)
                    at_step(pidx + 3, do_bcast)
                    at_step(pidx + 4, do_mult)
                    if h == HPC - 1:
                        at_step(pidx + 4, lambda g=g: out_proj(g))

            emits = [lambda: qkT_group(1, xTb_t)]
            emits += [(lambda t=t: v_proj(t, xTb_t)) for t in range(8, NT)]
            n_emit = len(emits)
            total_ticks = EARLY_N + n_emit + (len(steps) - EARLY_N) \
                + DELAY + 10
            a_i = 0
            b_i = 0
            for tick in range(total_ticks):
                emit_phase = EARLY_N <= tick < EARLY_N + n_emit
                if tick < EARLY_N:
                    stage_a(a_i)
                    a_i += 1
                elif emit_phase:
                    emits[tick - EARLY_N]()
                elif a_i < len(steps):
                    stage_a(a_i)
                    a_i += 1
                if b_i < a_i and (a_i - b_i > DELAY or emit_phase
                                  or a_i == len(steps)):
                    stage_b(b_i, tick)
                    b_i += 1
                for fn in post.pop(tick, []):
                    fn()

    nc.compile()
    from concourse.bass_interp import get_hw_module

    nc.m = get_hw_module(nc.m)
    return nc


def _shard_inputs(x, Wqkv, bqkv, Wo, bo):
    import ml_dtypes

    bfdt = ml_dtypes.bfloat16

    x = np.asarray(x, np.float32)
    Wqkv = np.asarray(Wqkv, np.float32)
    bqkv = np.asarray(bqkv, np.float32)
    Wo = np.asarray(Wo, np.float32)

    scale = 1.0 / np.sqrt(np.float32(HD))
    c_idx = np.arange(128)[:, None]
    u_idx = np.arange(128)[None, :]
    tri0 = (u_idx >= c_idx).astype(np.float32)   # keys block j vs q block j
    tri1 = (u_idx < c_idx).astype(np.float32)    # keys block j vs q block j+2
    tris = np.concatenate([tri0, tri1], axis=1)

    in_maps = []
    for c in range(NCORES):
        b, hg = divmod(c, HPC)
        r0 = hg * FB
        Wq = Wqkv[r0:r0 + FB] * scale
        Wk = Wqkv[D + r0:D + r0 + FB]
        Wv = Wqkv[2 * D + r0:2 * D + r0 + FB]
        bq = bqkv[r0:r0 + FB] * scale
        bk = bqkv[D + r0:D + r0 + FB]
        bv = bqkv[2 * D + r0:2 * D + r0 + FB]
        in_maps.append({
            "xT": np.ascontiguousarray(x[b].T).astype(bfdt),
            "wqk": np.ascontiguousarray(
                np.concatenate([Wq, Wk], 0).T).astype(bfdt),
            "bqk": np.ascontiguousarray(
                np.concatenate([bq, bk]).reshape(4, 128).T),
            "wv": np.ascontiguousarray(Wv.T).astype(bfdt),
            "bvb": np.ascontiguousarray(
                np.broadcast_to(bv[None, :], (128, FB))
            ).reshape(128, HPC, HD),
            "wo": np.ascontiguousarray(Wo[:, r0:r0 + FB].T).astype(bfdt),
            "tris": tris.astype(bfdt),
            "vone": np.ones((128, 64), bfdt),
        })
    return in_maps


def kernel(x, Wqkv, bqkv, Wo, bo):
    from concourse import bass_utils

    if "nc" not in _STATE:
        _STATE["nc"] = _build_module()
    nc = _STATE["nc"]

    in_maps = _shard_inputs(x, Wqkv, bqkv, Wo, bo)
    trace = bool(os.environ.get("TRNKERN_TRACE"))
    res = bass_utils.run_bass_kernel_spmd(
        nc,
        in_maps,
        core_ids=list(range(NCORES)),
        trace=trace,
    )
    _STATE["last"] = res

    bo = np.asarray(bo, np.float32)
    out = np.empty((B, T, D), np.float32)
    for b in range(B):
        acc = res.results[b * HPC]["out_p"].astype(np.float32)
        for hg in range(1, HPC):
            acc = acc + res.results[b * HPC + hg]["out_p"].astype(np.float32)
        out[b] = acc + bo[None, :]
    return out
